# revision 16
# baseline (speedup 1.0000x reference)
"""Multi-head attention forward on 8 TRN2 NeuronCores.

Problem: x[2,2048,1024] @ {Wq,Wk,Wv}[1024,1024] (+bias) -> 16 heads of 64,
softmax(QK^T/8)V per head, concat -> @Wo[1024,1024] + bo.

Sharding: tensor-parallel over d_hid. Core c owns 2 heads (128 dims):
  - computes Q^T,K^T,V^T slices [128, 2048/batch] from full x^T
  - attention for its (2 batches x 2 heads)
  - partial out = ctx_slice @ Wo[slice_rows] -> [4096, 1024] in f16
Host sums the 8 partials and adds bo (pure reduction, no collectives).

v3 structure. The scalar engine's exp stream (~131us/core over 128
ACTIVATEs) is the hard floor; every other engine must hide under it:
  - scores matmuls run 2-heads-concurrent via 64x128 PE row tiling.
  - one exp ACTIVATE per (qh, ki): sc[128, 1024] spans 2 PSUM banks.
  - softmax denominator via ones-augmented V (cols 64/129 of vaug,
    written by memset - no input dependency the scheduler can hoist
    into a DVE head-of-line block).
  - batch-0 qh0 runs with DEFERRED ctx: K chunk 0 + Q chunk 0 load
    first, the 16 score/exp pairs stream from ~16us while the V
    projection + transposes + deferred ctx matmuls trail behind
    (et pool is 9 deep to hold the exp->ctx backlog).
  - cross-batch pipelining by emission order: batch-1 projections and
    V transposes hide inside batch-0's attention, batch-0 AND most of
    batch-1's out-projection inside batch-1's attention (out tiles for
    q-rows of qh become ready right after qh's normalization).
  - out-projection/out DMA all f16; tail out-unit PSUM evacuations are
    split between the scalar engine (idle after the last exp) and DVE.
  - DMA triggers ordered by need time: scalar queue carries biases+
    wk/wq/wv, sync carries batch-0 x^T, gpsimd carries idt + batch-1
    x^T + wo.
  - PSUM: sc double-buffer (4 banks) + ctx h0/h1 (2) + proj staging (2).
"""

import os
import numpy as np

B, S, D = 2, 2048, 1024
NCORES = 8
HSLICE = D // NCORES          # 128 = 2 heads x 64
KT = D // 128                 # 8 contraction tiles for projections
NKT = S // 128                # 16 k-tiles per batch for attention
QH = 512                      # q chunk (one PSUM bank per head)
CH = 512                      # matmul free-dim chunk

_cache = {}


def _build():
    import concourse.bacc as bacc
    import concourse.tile as tile
    from concourse import mybir

    f32 = mybir.dt.float32
    f32r = mybir.dt.float32r
    f16 = mybir.dt.float16
    AF = mybir.ActivationFunctionType

    nc = bacc.Bacc("TRN2", target_bir_lowering=False, debug=False,
                   num_devices=NCORES)

    xt_d = nc.dram_tensor("xt", [D, B * S], f16, kind="ExternalInput").ap()
    wq_d = nc.dram_tensor("wq", [D, HSLICE], f16, kind="ExternalInput").ap()
    wk_d = nc.dram_tensor("wk", [D, HSLICE], f16, kind="ExternalInput").ap()
    wv_d = nc.dram_tensor("wv", [D, HSLICE], f16, kind="ExternalInput").ap()
    bq_d = nc.dram_tensor("bq", [HSLICE, 1], f32, kind="ExternalInput").ap()
    bk_d = nc.dram_tensor("bk", [HSLICE, 1], f32, kind="ExternalInput").ap()
    bv_d = nc.dram_tensor("bv", [HSLICE, 1], f32, kind="ExternalInput").ap()
    wo_d = nc.dram_tensor("wo", [HSLICE, D], f16, kind="ExternalInput").ap()
    idt_d = nc.dram_tensor("idt", [128, 128], f32r, kind="ExternalInput").ap()
    out_d = nc.dram_tensor("out", [B * S, D], f16, kind="ExternalOutput").ap()

    with tile.TileContext(nc) as tc:
        with (
            tc.tile_pool(name="wpool", bufs=1) as wpool,
            tc.tile_pool(name="xt", bufs=1) as xtp,
            tc.tile_pool(name="qk", bufs=2) as qkp,
            tc.tile_pool(name="vtmp", bufs=2) as vtp,
            tc.tile_pool(name="vaug", bufs=2) as vap,
            tc.tile_pool(name="et", bufs=9) as etp,
            tc.tile_pool(name="ctx", bufs=2) as ctxp,
            tc.tile_pool(name="norm", bufs=2) as normp,
            tc.tile_pool(name="ost", bufs=3) as ostp,
            tc.tile_pool(name="psS", bufs=2, space="PSUM") as psS,
            tc.tile_pool(name="psC", bufs=1, space="PSUM") as psC,
            tc.tile_pool(name="psP", bufs=2, space="PSUM") as psP,
        ):
            # ---- small inputs on the scalar HWDGE queue, ordered by
            # first use; big streams elsewhere ----
            bk_t = wpool.tile([128, 1], f32, tag="bk")
            nc.scalar.dma_start(bk_t[:], bk_d[:])
            bq_t = wpool.tile([128, 1], f32, tag="bq")
            nc.scalar.dma_start(bq_t[:], bq_d[:])
            bv_t = wpool.tile([128, 1], f32, tag="bv")
            nc.scalar.dma_start(bv_t[:], bv_d[:])
            wq_t, wk_t, wv_t = [], [], []
            for lst, src, tag in ((wk_t, wk_d, "wk"), (wq_t, wq_d, "wq"),
                                  (wv_t, wv_d, "wv")):
                for ki in range(KT):
                    t = wpool.tile([128, HSLICE], f16, tag=f"{tag}{ki}")
                    nc.scalar.dma_start(t[:], src[ki * 128:(ki + 1) * 128, :])
                    lst.append(t)
            idt = wpool.tile([128, 128], f32r, tag="idt")
            nc.gpsimd.dma_start(idt[:], idt_d[:])

            # x^T batch-0 on the sync queue in two quarter-waves.
            xts = [xtp.tile([128, B * S], f16, tag=f"xt{ki}",
                            name=f"xt{ki}")
                   for ki in range(KT)]
            for cs in (slice(0, S // 2), slice(S // 2, S)):
                for ki in range(KT):
                    nc.sync.dma_start(xts[ki][:, cs],
                                      xt_d[ki * 128:(ki + 1) * 128, cs])
            # batch-1 x^T + wo are NOT needed before ~70us; gate them
            # behind batch-0's K chunk 0 so their 4.25MB doesn't steal
            # HBM bandwidth from the critical head loads (DMA rings
            # round-robin all in-flight transfers). The [1,1] SBUF copy
            # below picks up a dependency on the K c0 bias-add; the
            # gpsimd queue is FIFO so everything after it waits too.
            b1_gate = wpool.tile([1, 1], f16, tag="b1gate")

            qt, ktl, vt, vaugs, ctxT = {}, {}, {}, {}, {}

            def proj_chunk(b, dst, w_t, b_t, c):
                """dst[:, c*CH:(c+1)*CH] = W^T @ x + bias for batch b."""
                ps = psP.tile([128, CH], f32, tag="pp")
                for ki in range(KT):
                    nc.tensor.matmul(ps[:], w_t[ki][:],
                                     xts[ki][:, b * S + c * CH:
                                             b * S + (c + 1) * CH],
                                     start=(ki == 0), stop=(ki == KT - 1))
                nc.vector.tensor_scalar_add(
                    dst[:, c * CH:(c + 1) * CH], ps[:], b_t[:, 0:1])

            def vtrans(b, ki):
                """vaugs[b][ki] [128,130]: V rows for k-tile ki, ones at
                cols 64/129 (softmax denominator trick)."""
                va = vap.tile([128, 130], f16, tag=f"va{ki}",
                              name=f"va{b}_{ki}")
                ps = psP.tile([128, 128], f32r, tag="pp")
                nc.tensor.transpose(ps[:], vt[b][:, ki * 128:(ki + 1) * 128],
                                    idt[:])
                nc.vector.tensor_copy(va[:, 0:64], ps[:, 0:64])
                nc.vector.tensor_copy(va[:, 65:129], ps[:, 64:128])
                nc.vector.memset(va[:, 64:65], 1.0)
                nc.vector.memset(va[:, 129:130], 1.0)
                vaugs[b][ki] = va

            def out_unit(b, st, evac=None):
                """Partial out rows for token tile st of batch b (f16)."""
                s0 = b * S
                ot = ostp.tile([128, D], f16, tag="ost")
                for c in range(D // CH):
                    ps = psP.tile([128, CH], f32, tag="pp")
                    nc.tensor.matmul(ps[:],
                                     ctxT[b][:, st * 128:(st + 1) * 128],
                                     wo_t[:, c * CH:(c + 1) * CH])
                    eng = evac(c) if evac else nc.vector.tensor_copy
                    eng(ot[:, c * CH:(c + 1) * CH], ps[:])
                nc.sync.dma_start(
                    out_d[s0 + st * 128:s0 + (st + 1) * 128, :], ot[:])

            def emit_scores_exp(b, qh, kis):
                """Row-tiled score matmuls + one exp per k-tile. Returns
                [(ki, et)] for the ctx stage."""
                q0 = qh * QH
                out = []
                for ki in kis:
                    sc = psS.tile([128, 2 * QH], f32, tag="sc")
                    for h in (0, 1):
                        hp = h * 64
                        nc.tensor.matmul(
                            sc[:, h * QH:(h + 1) * QH],
                            ktl[b][hp:hp + 64, ki * 128:(ki + 1) * 128],
                            qt[b][hp:hp + 64, q0:q0 + QH])
                    et = etp.tile([128, 2 * QH], f16, tag="et")
                    nc.scalar.activation(et[:], sc[:], AF.Exp)
                    out.append((ki, et))
                return out

            def emit_ctx(b, ctx_ps, pairs):
                for ki, et in pairs:
                    for h in (0, 1):
                        nc.tensor.matmul(
                            ctx_ps[h][:],
                            vaugs[b][ki][:, h * 65:h * 65 + 65],
                            et[:, h * QH:(h + 1) * QH],
                            start=(ki == 0), stop=(ki == NKT - 1))

            def emit_norm(b, qh, ctx_ps):
                """ctxT[b][:, qh block] = ctx / (ones-row sums)."""
                q0 = qh * QH
                for h in range(2):
                    hp = h * 64
                    stg = normp.tile([128, QH], f32, tag=f"stg{h}")
                    nc.vector.tensor_copy(stg[0:65, :], ctx_ps[h][0:65, :])
                    r0 = normp.tile([1, QH], f32, tag="r0")
                    nc.gpsimd.dma_start(r0[:], stg[64:65, :])
                    bcs = normp.tile([64, QH], f32, tag="bcs")
                    nc.gpsimd.partition_broadcast(bcs[:], r0[:])
                    bc = normp.tile([64, QH], f32, tag="bc")
                    scr = normp.tile([64, QH], f32, tag="scr")
                    nc.vector.reciprocal_approx_accurate(
                        bc[:], bcs[:], scratch=scr[:])
                    nc.vector.tensor_mul(
                        out=ctxT[b][hp:hp + 64, q0:q0 + QH],
                        in0=stg[0:64, :], in1=bc[:])

            def ctx_tiles():
                return [psC.tile([65, QH], f32, tag="ctx0", name="ctx0"),
                        psC.tile([65, QH], f32, tag="ctx1", name="ctx1")]

            def attn_qh(b, qh, extras):
                """Steady-state attention for one q block; extras are
                zero-arg emitters of filler PE/DVE work, consumed one
                per k-tile pair. The last pair stays extra-free and any
                leftovers go AFTER the norm so its DVE stage copies are
                not stuck behind extra work in the DVE FIFO (they free
                the ctx PSUM banks for the next q block)."""
                ctx_ps = ctx_tiles()
                ex = list(extras)
                for kp in range(NKT // 2):
                    pairs = emit_scores_exp(b, qh, (2 * kp, 2 * kp + 1))
                    emit_ctx(b, ctx_ps, pairs)
                    if ex and kp < 7:
                        ex.pop(0)()
                emit_norm(b, qh, ctx_ps)
                for fn in ex:
                    fn()

            def fresh_batch_tiles(b):
                qt[b] = qkp.tile([128, S], f16, tag="qt", name=f"qt{b}")
                ktl[b] = qkp.tile([128, S], f16, tag="kt", name=f"kt{b}")
                vt[b] = vtp.tile([128, S], f32r, tag="vt", name=f"vt{b}")
                vaugs[b] = [None] * NKT
                ctxT[b] = ctxp.tile([128, S], f16, tag="ctxT",
                                    name=f"ctxT{b}")

            # ================= emission schedule =================
            fresh_batch_tiles(0)
            fresh_batch_tiles(1)

            # --- batch-0 qh0 with deferred ctx: exp stream starts right
            # after K chunk 0 + Q chunk 0; V/transposes/ctx trail. ---
            proj_chunk(0, ktl[0], wk_t, bk_t, 0)
            nc.gpsimd.dma_start(b1_gate[:], ktl[0][0:1, 0:1])
            for ki in range(KT):
                nc.gpsimd.dma_start(xts[ki][:, S:2 * S],
                                    xt_d[ki * 128:(ki + 1) * 128, S:2 * S])
            wo_t = wpool.tile([128, D], f16, tag="wo")
            nc.gpsimd.dma_start(wo_t[:], wo_d[:])
            proj_chunk(0, qt[0], wq_t, bq_t, 0)
            ctx0_ps = ctx_tiles()
            backlog = emit_scores_exp(0, 0, (0, 1))
            proj_chunk(0, ktl[0], wk_t, bk_t, 1)
            backlog += emit_scores_exp(0, 0, (2, 3))
            proj_chunk(0, vt[0], wv_t, bv_t, 0)
            backlog += emit_scores_exp(0, 0, (4, 5))
            for t in range(4):
                vtrans(0, t)
            backlog += emit_scores_exp(0, 0, (6, 7))
            emit_ctx(0, ctx0_ps, backlog[0:4])
            proj_chunk(0, ktl[0], wk_t, bk_t, 2)
            backlog += emit_scores_exp(0, 0, (8, 9))
            proj_chunk(0, vt[0], wv_t, bv_t, 1)
            backlog += emit_scores_exp(0, 0, (10, 11))
            for t in range(4, 8):
                vtrans(0, t)
            emit_ctx(0, ctx0_ps, backlog[4:8])
            proj_chunk(0, ktl[0], wk_t, bk_t, 3)
            backlog += emit_scores_exp(0, 0, (12, 13))
            proj_chunk(0, vt[0], wv_t, bv_t, 2)
            backlog += emit_scores_exp(0, 0, (14, 15))
            for t in range(8, 12):
                vtrans(0, t)
            emit_ctx(0, ctx0_ps, backlog[8:12])
            proj_chunk(0, qt[0], wq_t, bq_t, 1)
            proj_chunk(0, vt[0], wv_t, bv_t, 3)
            for t in range(12, 16):
                vtrans(0, t)
            emit_ctx(0, ctx0_ps, backlog[12:16])
            emit_norm(0, 0, ctx0_ps)

            # --- batch-0 qh1-3; batch-1 proj work hides in the slack ---
            b1_work = [
                lambda c=c: proj_chunk(1, ktl[1], wk_t, bk_t, c)
                for c in range(4)
            ] + [
                lambda c=c: (proj_chunk(1, vt[1], wv_t, bv_t, c),
                             [vtrans(1, 4 * c + t) for t in range(4)])
                for c in range(4)
            ] + [
                lambda: proj_chunk(1, qt[1], wq_t, bq_t, 0),
            ]
            for qh in range(1, 4):
                extras = []
                if qh < 3:
                    extras.append(
                        lambda c=qh + 1: proj_chunk(0, qt[0], wq_t, bq_t, c))
                extras += [b1_work.pop(0) for _ in range(3)]
                attn_qh(0, qh, extras)

            # --- batch-1 attention; leftover b1 proj, b1 Q chunks and
            # both batches' out-projections hide in the slack ---
            # out rows for (b, st) are final after batch b's qh=st//4
            # norm; spread all 32 units as early as legality allows.
            unit_sched = {
                0: [(0, st) for st in range(0, 8)],
                1: [(0, st) for st in range(8, 14)] + [(1, 0), (1, 1)],
                2: [(0, 14), (0, 15)] + [(1, st) for st in range(2, 8)],
                3: [(1, st) for st in range(8, 12)],
            }
            for qh in range(4):
                extras = list(b1_work)
                b1_work = []
                if qh < 3:
                    extras.append(
                        lambda c=qh + 1: proj_chunk(1, qt[1], wq_t, bq_t, c))
                extras += [lambda u=u: out_unit(u[0], u[1])
                           for u in unit_sched[qh]]
                attn_qh(1, qh, extras)

            # --- tail: last batch-1 out rows; split PSUM evacuation
            # between the (now idle) scalar engine and DVE ---
            def tail_evac(c):
                return nc.scalar.copy if c == 0 else nc.vector.tensor_copy
            for st in (12, 13, 14, 15):
                out_unit(1, st, evac=tail_evac)

    nc.compile()
    return nc


def _get_nc():
    if "nc" not in _cache:
        _cache["nc"] = _build()
    return _cache["nc"]


def kernel(x, Wq, bq, Wk, bk, Wv, bv, Wo, bo):
    from concourse.bass_utils import run_bass_kernel_spmd

    nc = _get_nc()

    x = np.ascontiguousarray(np.asarray(x, dtype=np.float32))
    xt = np.ascontiguousarray(x.reshape(B * S, D).T)          # [D, B*S]
    idt = np.eye(128, dtype=np.float32)

    in_maps = []
    for c in range(NCORES):
        sl = slice(c * HSLICE, (c + 1) * HSLICE)
        in_maps.append({
            "xt": xt.astype(np.float16),
            "wq": (np.ascontiguousarray(np.asarray(Wq, np.float32)[:, sl]) / 8.0).astype(np.float16),
            "wk": np.ascontiguousarray(np.asarray(Wk, np.float32)[:, sl]).astype(np.float16),
            "wv": np.ascontiguousarray(np.asarray(Wv, np.float32)[:, sl]).astype(np.float16),
            "bq": (np.asarray(bq, np.float32)[sl] / 8.0).reshape(HSLICE, 1),
            "bk": np.asarray(bk, np.float32)[sl].reshape(HSLICE, 1),
            "bv": np.asarray(bv, np.float32)[sl].reshape(HSLICE, 1),
            "wo": np.ascontiguousarray(np.asarray(Wo, np.float32)[sl, :]).astype(np.float16),
            "idt": idt,
        })

    res = run_bass_kernel_spmd(nc, in_maps, core_ids=list(range(NCORES)),
                               trace=bool(int(os.environ.get("KTRACE", "0"))))
    _cache["last_result"] = res
    acc = res.results[0]["out"].astype(np.float32)
    for c in range(1, NCORES):
        acc += res.results[c]["out"].astype(np.float32)
    acc += np.asarray(bo, np.float32)[None, :]
    return acc.reshape(B, S, D)


# revision 22
# speedup vs baseline: 1.0142x; 1.0142x over previous
"""Multi-head attention forward on 8 TRN2 NeuronCores.

Problem: x[2,2048,1024] @ {Wq,Wk,Wv}[1024,1024] (+bias) -> 16 heads of 64,
softmax(QK^T/8)V per head, concat -> @Wo[1024,1024] + bo.

Sharding: tensor-parallel over d_hid. Core c owns 2 heads (128 dims):
  - computes Q^T,K^T,V^T slices [128, 2048/batch] from full x^T
  - attention for its (2 batches x 2 heads)
  - partial out = ctx_slice @ Wo[slice_rows] -> [4096, 1024] in f16
Host sums the 8 partials and adds bo (pure reduction, no collectives).

v3 structure. The scalar engine's exp stream (~131us/core over 128
ACTIVATEs) is the hard floor; every other engine must hide under it:
  - scores matmuls run 2-heads-concurrent via 64x128 PE row tiling.
  - one exp ACTIVATE per (qh, ki): sc[128, 1024] spans 2 PSUM banks.
  - softmax denominator via ones-augmented V (cols 64/129 of vaug,
    written by memset - no input dependency the scheduler can hoist
    into a DVE head-of-line block).
  - batch-0 qh0 runs with DEFERRED ctx: K chunk 0 + Q chunk 0 load
    first, the 16 score/exp pairs stream from ~16us while the V
    projection + transposes + deferred ctx matmuls trail behind
    (et pool is 9 deep to hold the exp->ctx backlog).
  - cross-batch pipelining by emission order: batch-1 projections and
    V transposes hide inside batch-0's attention, batch-0 AND most of
    batch-1's out-projection inside batch-1's attention (out tiles for
    q-rows of qh become ready right after qh's normalization).
  - out-projection/out DMA all f16; tail out-unit PSUM evacuations are
    split between the scalar engine (idle after the last exp) and DVE.
  - DMA triggers ordered by need time: scalar queue carries biases+
    wk/wq/wv, sync carries batch-0 x^T, gpsimd carries idt + batch-1
    x^T + wo.
  - PSUM: sc double-buffer (4 banks) + ctx h0/h1 (2) + proj staging (2).
"""

import os
import numpy as np

B, S, D = 2, 2048, 1024
NCORES = 8
HSLICE = D // NCORES          # 128 = 2 heads x 64
KT = D // 128                 # 8 contraction tiles for projections
NKT = S // 128                # 16 k-tiles per batch for attention
QH = 512                      # q chunk (one PSUM bank per head)
CH = 512                      # matmul free-dim chunk

_cache = {}


def _build():
    import concourse.bacc as bacc
    import concourse.tile as tile
    from concourse import mybir

    f32 = mybir.dt.float32
    f32r = mybir.dt.float32r
    f16 = mybir.dt.float16
    AF = mybir.ActivationFunctionType

    nc = bacc.Bacc("TRN2", target_bir_lowering=False, debug=False,
                   num_devices=NCORES)

    # x^T pre-packed on the host so every DMA tile is one contiguous
    # DRAM block (strided 2KB/partition-line reads only sustain
    # ~115GB/s; contiguous blocks run near the ~358GB/s HBM limit).
    # xt0: batch-0 as two column-waves of 8 k-tiles [128,1024];
    # xt1: batch-1 as 8 k-tiles [128,2048].
    xt0_d = nc.dram_tensor("xt0", [2 * KT * 128, S // 2], f16,
                           kind="ExternalInput").ap()
    xt1_d = nc.dram_tensor("xt1", [KT * 128, S], f16,
                           kind="ExternalInput").ap()
    wq_d = nc.dram_tensor("wq", [D, HSLICE], f16, kind="ExternalInput").ap()
    wk_d = nc.dram_tensor("wk", [D, HSLICE], f16, kind="ExternalInput").ap()
    wv_d = nc.dram_tensor("wv", [D, HSLICE], f16, kind="ExternalInput").ap()
    bq_d = nc.dram_tensor("bq", [HSLICE, 1], f32, kind="ExternalInput").ap()
    bk_d = nc.dram_tensor("bk", [HSLICE, 1], f32, kind="ExternalInput").ap()
    bv_d = nc.dram_tensor("bv", [HSLICE, 1], f32, kind="ExternalInput").ap()
    wo_d = nc.dram_tensor("wo", [HSLICE, D], f16, kind="ExternalInput").ap()
    idt_d = nc.dram_tensor("idt", [128, 128], f32r, kind="ExternalInput").ap()
    out_d = nc.dram_tensor("out", [B * S, D], f16, kind="ExternalOutput").ap()

    with tile.TileContext(nc) as tc:
        with (
            tc.tile_pool(name="wpool", bufs=1) as wpool,
            tc.tile_pool(name="xt", bufs=1) as xtp,
            tc.tile_pool(name="qk", bufs=2) as qkp,
            tc.tile_pool(name="vtmp", bufs=2) as vtp,
            tc.tile_pool(name="vaug", bufs=2) as vap,
            tc.tile_pool(name="et", bufs=9) as etp,
            tc.tile_pool(name="ctx", bufs=2) as ctxp,
            tc.tile_pool(name="norm", bufs=2) as normp,
            tc.tile_pool(name="ost", bufs=3) as ostp,
            tc.tile_pool(name="psS", bufs=2, space="PSUM") as psS,
            tc.tile_pool(name="psC", bufs=1, space="PSUM") as psC,
            tc.tile_pool(name="psP", bufs=2, space="PSUM") as psP,
        ):
            # ---- small inputs on the scalar HWDGE queue, ordered by
            # first use; big streams elsewhere ----
            bk_t = wpool.tile([128, 1], f32, tag="bk")
            nc.scalar.dma_start(bk_t[:], bk_d[:])
            bq_t = wpool.tile([128, 1], f32, tag="bq")
            nc.scalar.dma_start(bq_t[:], bq_d[:])
            bv_t = wpool.tile([128, 1], f32, tag="bv")
            nc.scalar.dma_start(bv_t[:], bv_d[:])
            wq_t, wk_t, wv_t = [], [], []
            for lst, src, tag in ((wk_t, wk_d, "wk"), (wq_t, wq_d, "wq"),
                                  (wv_t, wv_d, "wv")):
                for ki in range(KT):
                    t = wpool.tile([128, HSLICE], f16, tag=f"{tag}{ki}")
                    nc.scalar.dma_start(t[:], src[ki * 128:(ki + 1) * 128, :])
                    lst.append(t)
            idt = wpool.tile([128, 128], f32r, tag="idt")
            nc.gpsimd.dma_start(idt[:], idt_d[:])

            # x^T batch-0 on the sync queue in two quarter-waves.
            xts = [xtp.tile([128, B * S], f16, tag=f"xt{ki}",
                            name=f"xt{ki}")
                   for ki in range(KT)]
            for w in range(2):
                for ki in range(KT):
                    r0 = (w * KT + ki) * 128
                    nc.sync.dma_start(
                        xts[ki][:, w * (S // 2):(w + 1) * (S // 2)],
                        xt0_d[r0:r0 + 128, :])
            # batch-1 x^T + wo are NOT needed before ~70us; gate them
            # behind batch-0's K chunk 0 so their 4.25MB doesn't steal
            # HBM bandwidth from the critical head loads (DMA rings
            # round-robin all in-flight transfers). The [1,1] SBUF copy
            # below picks up a dependency on the K c0 bias-add; the
            # gpsimd queue is FIFO so everything after it waits too.
            b1_gate = wpool.tile([1, 1], f16, tag="b1gate")

            qt, ktl, vt, vaugs, ctxT = {}, {}, {}, {}, {}

            def proj_chunk(b, dst, w_t, b_t, c):
                """dst[:, c*CH:(c+1)*CH] = W^T @ x + bias for batch b."""
                ps = psP.tile([128, CH], f32, tag="pp")
                for ki in range(KT):
                    nc.tensor.matmul(ps[:], w_t[ki][:],
                                     xts[ki][:, b * S + c * CH:
                                             b * S + (c + 1) * CH],
                                     start=(ki == 0), stop=(ki == KT - 1))
                nc.vector.tensor_scalar_add(
                    dst[:, c * CH:(c + 1) * CH], ps[:], b_t[:, 0:1])

            def vtrans(b, ki):
                """vaugs[b][ki] [128,130]: V rows for k-tile ki, ones at
                cols 64/129 (softmax denominator trick)."""
                va = vap.tile([128, 130], f16, tag=f"va{ki}",
                              name=f"va{b}_{ki}")
                ps = psP.tile([128, 128], f32r, tag="pp")
                nc.tensor.transpose(ps[:], vt[b][:, ki * 128:(ki + 1) * 128],
                                    idt[:])
                nc.vector.tensor_copy(va[:, 0:64], ps[:, 0:64])
                nc.vector.tensor_copy(va[:, 65:129], ps[:, 64:128])
                nc.vector.memset(va[:, 64:65], 1.0)
                nc.vector.memset(va[:, 129:130], 1.0)
                vaugs[b][ki] = va

            def out_unit(b, st, evac=None):
                """Partial out rows for token tile st of batch b (f16)."""
                s0 = b * S
                ot = ostp.tile([128, D], f16, tag="ost")
                for c in range(D // CH):
                    ps = psP.tile([128, CH], f32, tag="pp")
                    nc.tensor.matmul(ps[:],
                                     ctxT[b][:, st * 128:(st + 1) * 128],
                                     wo_t[:, c * CH:(c + 1) * CH])
                    eng = evac(c) if evac else nc.vector.tensor_copy
                    eng(ot[:, c * CH:(c + 1) * CH], ps[:])
                nc.sync.dma_start(
                    out_d[s0 + st * 128:s0 + (st + 1) * 128, :], ot[:])

            def emit_scores_exp(b, qh, kis):
                """Row-tiled score matmuls + one exp per k-tile. Returns
                [(ki, et)] for the ctx stage."""
                q0 = qh * QH
                out = []
                for ki in kis:
                    sc = psS.tile([128, 2 * QH], f32, tag="sc")
                    for h in (0, 1):
                        hp = h * 64
                        nc.tensor.matmul(
                            sc[:, h * QH:(h + 1) * QH],
                            ktl[b][hp:hp + 64, ki * 128:(ki + 1) * 128],
                            qt[b][hp:hp + 64, q0:q0 + QH])
                    et = etp.tile([128, 2 * QH], f16, tag="et")
                    nc.scalar.activation(et[:], sc[:], AF.Exp)
                    out.append((ki, et))
                return out

            def emit_ctx(b, ctx_ps, pairs):
                for ki, et in pairs:
                    for h in (0, 1):
                        nc.tensor.matmul(
                            ctx_ps[h][:],
                            vaugs[b][ki][:, h * 65:h * 65 + 65],
                            et[:, h * QH:(h + 1) * QH],
                            start=(ki == 0), stop=(ki == NKT - 1))

            def emit_norm(b, qh, ctx_ps):
                """ctxT[b][:, qh block] = ctx / (ones-row sums)."""
                q0 = qh * QH
                for h in range(2):
                    hp = h * 64
                    stg = normp.tile([128, QH], f32, tag=f"stg{h}")
                    nc.vector.tensor_copy(stg[0:65, :], ctx_ps[h][0:65, :])
                    r0 = normp.tile([1, QH], f32, tag="r0")
                    nc.gpsimd.dma_start(r0[:], stg[64:65, :])
                    bcs = normp.tile([64, QH], f32, tag="bcs")
                    nc.gpsimd.partition_broadcast(bcs[:], r0[:])
                    bc = normp.tile([64, QH], f32, tag="bc")
                    scr = normp.tile([64, QH], f32, tag="scr")
                    nc.vector.reciprocal_approx_accurate(
                        bc[:], bcs[:], scratch=scr[:])
                    nc.vector.tensor_mul(
                        out=ctxT[b][hp:hp + 64, q0:q0 + QH],
                        in0=stg[0:64, :], in1=bc[:])

            def ctx_tiles():
                return [psC.tile([65, QH], f32, tag="ctx0", name="ctx0"),
                        psC.tile([65, QH], f32, tag="ctx1", name="ctx1")]

            def attn_qh(b, qh, extras, pre=()):
                """Steady-state attention for one q block; extras are
                zero-arg emitters of filler PE/DVE work, consumed one
                per k-tile pair. The last pair stays extra-free and any
                leftovers go AFTER the norm so its DVE stage copies are
                not stuck behind extra work in the DVE FIFO (they free
                the ctx PSUM banks for the next q block). `pre` carries
                (ki, et) pairs whose scores/exp were already emitted by
                the previous block (keeps ACT fed across the junction)."""
                ctx_ps = ctx_tiles()
                ex = list(extras)
                pre = list(pre)
                if pre:
                    emit_ctx(b, ctx_ps, pre)
                for kp in range(len(pre) // 2, NKT // 2):
                    pairs = emit_scores_exp(b, qh, (2 * kp, 2 * kp + 1))
                    emit_ctx(b, ctx_ps, pairs)
                    if ex and kp < 7:
                        ex.pop(0)()
                emit_norm(b, qh, ctx_ps)
                for fn in ex:
                    fn()

            def fresh_batch_tiles(b):
                qt[b] = qkp.tile([128, S], f16, tag="qt", name=f"qt{b}")
                ktl[b] = qkp.tile([128, S], f16, tag="kt", name=f"kt{b}")
                vt[b] = vtp.tile([128, S], f32r, tag="vt", name=f"vt{b}")
                vaugs[b] = [None] * NKT
                ctxT[b] = ctxp.tile([128, S], f16, tag="ctxT",
                                    name=f"ctxT{b}")

            # ================= emission schedule =================
            fresh_batch_tiles(0)
            fresh_batch_tiles(1)

            # --- batch-0 qh0 with deferred ctx: exp stream starts right
            # after K chunk 0 + Q chunk 0; V/transposes/ctx trail. ---
            proj_chunk(0, ktl[0], wk_t, bk_t, 0)
            nc.gpsimd.dma_start(b1_gate[:], ktl[0][0:1, 0:1])
            for ki in range(KT):
                nc.gpsimd.dma_start(xts[ki][:, S:2 * S],
                                    xt1_d[ki * 128:(ki + 1) * 128, :])
            wo_t = wpool.tile([128, D], f16, tag="wo")
            nc.gpsimd.dma_start(wo_t[:], wo_d[:])
            proj_chunk(0, qt[0], wq_t, bq_t, 0)
            ctx0_ps = ctx_tiles()
            backlog = emit_scores_exp(0, 0, (0, 1))
            proj_chunk(0, ktl[0], wk_t, bk_t, 1)
            backlog += emit_scores_exp(0, 0, (2, 3))
            proj_chunk(0, vt[0], wv_t, bv_t, 0)
            backlog += emit_scores_exp(0, 0, (4, 5))
            for t in range(4):
                vtrans(0, t)
            backlog += emit_scores_exp(0, 0, (6, 7))
            emit_ctx(0, ctx0_ps, backlog[0:4])
            proj_chunk(0, ktl[0], wk_t, bk_t, 2)
            backlog += emit_scores_exp(0, 0, (8, 9))
            proj_chunk(0, vt[0], wv_t, bv_t, 1)
            backlog += emit_scores_exp(0, 0, (10, 11))
            for t in range(4, 8):
                vtrans(0, t)
            emit_ctx(0, ctx0_ps, backlog[4:8])
            proj_chunk(0, ktl[0], wk_t, bk_t, 3)
            backlog += emit_scores_exp(0, 0, (12, 13))
            proj_chunk(0, vt[0], wv_t, bv_t, 2)
            backlog += emit_scores_exp(0, 0, (14, 15))
            for t in range(8, 12):
                vtrans(0, t)
            emit_ctx(0, ctx0_ps, backlog[8:12])
            proj_chunk(0, qt[0], wq_t, bq_t, 1)
            proj_chunk(0, vt[0], wv_t, bv_t, 3)
            for t in range(12, 16):
                vtrans(0, t)
            pre1 = emit_scores_exp(0, 1, (0, 1))
            emit_ctx(0, ctx0_ps, backlog[12:16])
            emit_norm(0, 0, ctx0_ps)

            # --- batch-0 qh1-3; batch-1 proj work hides in the slack.
            # V transposes are separate lambdas so they land a slot
            # after their V chunk (bias-add already drained). ---
            b1_work = [
                lambda c=c: proj_chunk(1, ktl[1], wk_t, bk_t, c)
                for c in range(4)
            ]
            for c in range(4):
                b1_work.append(
                    lambda c=c: proj_chunk(1, vt[1], wv_t, bv_t, c))
                b1_work.append(
                    lambda c=c: [vtrans(1, 4 * c + t) for t in range(4)])
            b1_work.append(lambda: proj_chunk(1, qt[1], wq_t, bq_t, 0))
            for qh in range(1, 4):
                extras = []
                if qh < 3:
                    extras.append(
                        lambda c=qh + 1: proj_chunk(0, qt[0], wq_t, bq_t, c))
                take = 4 if qh < 3 else 5
                extras += [b1_work.pop(0) for _ in range(take)]
                attn_qh(0, qh, extras, pre=pre1 if qh == 1 else ())

            # --- batch-1 attention; leftover b1 proj, b1 Q chunks and
            # both batches' out-projections hide in the slack ---
            # out rows for (b, st) are final after batch b's qh=st//4
            # norm; spread all 32 units as early as legality allows.
            unit_sched = {
                0: [(0, st) for st in range(0, 8)],
                1: [(0, st) for st in range(8, 14)] + [(1, 0), (1, 1)],
                2: [(0, 14), (0, 15)] + [(1, st) for st in range(2, 8)],
                3: [(1, st) for st in range(8, 12)],
            }
            for qh in range(4):
                extras = list(b1_work)
                b1_work = []
                if qh < 3:
                    extras.append(
                        lambda c=qh + 1: proj_chunk(1, qt[1], wq_t, bq_t, c))
                extras += [lambda u=u: out_unit(u[0], u[1])
                           for u in unit_sched[qh]]
                attn_qh(1, qh, extras)

            # --- tail: last batch-1 out rows; split PSUM evacuation
            # between the (now idle) scalar engine and DVE ---
            def tail_evac(c):
                return nc.scalar.copy if c == 0 else nc.vector.tensor_copy
            for st in (12, 13, 14, 15):
                out_unit(1, st, evac=tail_evac)

    nc.compile()
    return nc


def _get_nc():
    if "nc" not in _cache:
        _cache["nc"] = _build()
    return _cache["nc"]


def kernel(x, Wq, bq, Wk, bk, Wv, bv, Wo, bo):
    from concourse.bass_utils import run_bass_kernel_spmd

    nc = _get_nc()

    x = np.ascontiguousarray(np.asarray(x, dtype=np.float32))
    xt = x.reshape(B * S, D).T.astype(np.float16)             # [D, B*S]
    # pack so each DMA tile is contiguous: xt0 = batch-0 in two
    # 1024-col waves of 8 k-tiles, xt1 = batch-1 as 8 k-tiles
    xt0 = np.ascontiguousarray(
        xt[:, :S].reshape(KT, 128, 2, S // 2).transpose(2, 0, 1, 3)
    ).reshape(2 * KT * 128, S // 2)
    xt1 = np.ascontiguousarray(xt[:, S:]).reshape(KT * 128, S)
    idt = np.eye(128, dtype=np.float32)

    in_maps = []
    for c in range(NCORES):
        sl = slice(c * HSLICE, (c + 1) * HSLICE)
        in_maps.append({
            "xt0": xt0,
            "xt1": xt1,
            "wq": (np.ascontiguousarray(np.asarray(Wq, np.float32)[:, sl]) / 8.0).astype(np.float16),
            "wk": np.ascontiguousarray(np.asarray(Wk, np.float32)[:, sl]).astype(np.float16),
            "wv": np.ascontiguousarray(np.asarray(Wv, np.float32)[:, sl]).astype(np.float16),
            "bq": (np.asarray(bq, np.float32)[sl] / 8.0).reshape(HSLICE, 1),
            "bk": np.asarray(bk, np.float32)[sl].reshape(HSLICE, 1),
            "bv": np.asarray(bv, np.float32)[sl].reshape(HSLICE, 1),
            "wo": np.ascontiguousarray(np.asarray(Wo, np.float32)[sl, :]).astype(np.float16),
            "idt": idt,
        })

    res = run_bass_kernel_spmd(nc, in_maps, core_ids=list(range(NCORES)),
                               trace=bool(int(os.environ.get("KTRACE", "0"))))
    _cache["last_result"] = res
    acc = res.results[0]["out"].astype(np.float32)
    for c in range(1, NCORES):
        acc += res.results[c]["out"].astype(np.float32)
    acc += np.asarray(bo, np.float32)[None, :]
    return acc.reshape(B, S, D)
